# revision 4
# baseline (speedup 1.0000x reference)
"""Bond-message embedding kernel for TRN2 (8 NeuronCores, SPMD).

Computes out[e, :] = concat(V[src[e]], E[e]) @ W.T + b for 800k edges.

Sharding: edges split evenly across the 8 cores (data-parallel over the
edge dim); V, W, b replicated; no cross-core comm. All device data is
bf16 (rel-err budget 2e-2; measured ~2.6e-3), which halves HBM traffic
vs f32.

Compute orientation: out.T = W @ concat(V[src], E).T, so the weights are
the PE's stationary operand (4 LDWEIGHTS per chunk) and the edge data
streams as the moving operand. The device writes out.T [256, per_core];
the host transposes it back.

Per-core device pipeline (per 1024-edge chunk):
  1. Load wrapped int16 gather indices [128, 64].
  2. dma_gather(transpose=True) pulls 1024 V rows (256B each) from HBM
     directly into feature-major layout vt [128 atom, 1024 edges].
  3. E arrives feature-major for free: the host ships E.T [64, per_core]
     bf16; one [64, 4KB] load covers two chunks.
  4. For msg-half h and edge-slice s (512): PSUM[128,512] = W1h.T-part
     matmul (vt slice) + accumulate W2h.T-part matmul (et slice).
  5. PSUM evac adds the per-partition bias and rounds to bf16 — half 0
     on the vector engine, half 1 on the scalar engine.
  6. One DMA stores both halves [128, 2x1024] to out.T rows (h*128+p).

Edges are reordered into a low group (src < 32768) and a high group so
the int16 gather indices stay in range (per-group HBM base offset); the
host undoes the reordering when it reassembles the full output.
"""

import ml_dtypes
import numpy as np

import concourse.bacc as bacc
import concourse.mybir as mybir
import concourse.tile as tile
from concourse.bass_utils import run_bass_kernel_spmd

F32 = mybir.dt.float32
BF16 = mybir.dt.bfloat16
I16 = mybir.dt.int16
NP_BF16 = ml_dtypes.bfloat16

N_CORES = 8
N_NODES = 50000
N_NODES_PAD = 50048           # 391 * 128
ATOM = 128
BOND = 64
MSG = 256
N_EDGES = 800000
SPLIT = 32768                 # int16-safe index boundary

PER_CORE_RAW = N_EDGES // N_CORES   # 100000
CHUNK = 1024
LOW_CHUNKS = 65               # 66560 low-group slots (expect ~65536)
HIGH_CHUNKS = 35              # 35840 high-group slots (expect ~34464)
N_CHUNKS = LOW_CHUNKS + HIGH_CHUNKS
LOW_SLOTS = LOW_CHUNKS * CHUNK
HIGH_SLOTS = HIGH_CHUNKS * CHUNK
PER_CORE = N_CHUNKS * CHUNK   # 102400 device slots per core

P = 128
IDX_COLS = CHUNK // 16        # 64


def _emit_pipeline(nc, tc, n_chunks, low_chunks, n_nodes_pad, split,
                   handles, reps=1, n_queues=1):
    v_h, e_h, idx_h, w1_h, w2_h, b_h, out_h = handles

    with (
        tc.tile_pool(name="const", bufs=1) as const,
        tc.tile_pool(name="chunkio", bufs=4) as chunkio,
        tc.tile_pool(name="epool", bufs=3) as epool,
        tc.tile_pool(name="work", bufs=3) as work,
        tc.tile_pool(name="psum", bufs=2, space="PSUM") as psum,
    ):
        # --- constants -----------------------------------------------------
        w1t = const.tile([ATOM, MSG], BF16)    # W.T rows 0:128 (atom part)
        nc.sync.dma_start(out=w1t[:], in_=w1_h[:, :])
        w2t = const.tile([BOND, MSG], BF16)    # W.T rows 128:192 (bond part)
        nc.sync.dma_start(out=w2t[:], in_=w2_h[:, :])
        b_t = const.tile([P, 2], F32)          # b_t[p, h] = b[h*128 + p]
        nc.sync.dma_start(out=b_t[:], in_=b_h[:, :])

        # --- main loop -----------------------------------------------------
        def chunk_body():
            for c2 in range(n_chunks // 2):
                e_t = epool.tile([BOND, 2 * CHUNK], BF16, tag="et")
                nc.sync.dma_start(
                    out=e_t[:], in_=e_h[:, 2 * c2 * CHUNK:(2 * c2 + 2) * CHUNK]
                )
                for ci in range(2):
                    c = 2 * c2 + ci
                    c0 = c * CHUNK
                    v_base = (
                        v_h[0:split, :] if c < low_chunks
                        else v_h[split:n_nodes_pad, :]
                    )

                    idx_t = chunkio.tile([P, IDX_COLS], I16, tag="idx")
                    nc.sync.dma_start(
                        out=idx_t[:], in_=idx_h[c * P:(c + 1) * P, :]
                    )
                    vt = chunkio.tile([P, CHUNK], BF16, tag="vt")
                    nc.gpsimd.dma_gather(
                        out_ap=vt[:].rearrange("p (o n) -> p o n", o=1),
                        in_ap=v_base,
                        idxs_ap=idx_t[:, :],
                        num_idxs=CHUNK,
                        num_idxs_reg=CHUNK,
                        elem_size=ATOM,
                        transpose=True,
                        single_packet=False,
                        queue_num=c % n_queues,
                    )

                    o_sb = work.tile([P, 2 * CHUNK], BF16, tag="osb")
                    for h in range(2):
                        ps = [None, None]
                        for s in range(2):
                            ps[s] = psum.tile([P, 512], F32, tag=f"ps{h}{s}",
                                              name=f"ps{h}{s}")
                            nc.tensor.matmul(
                                out=ps[s][:],
                                lhsT=w1t[:, h * P:(h + 1) * P],
                                rhs=vt[:, s * 512:(s + 1) * 512],
                                start=True,
                                stop=False,
                            )
                        for s in range(2):
                            nc.tensor.matmul(
                                out=ps[s][:],
                                lhsT=w2t[:, h * P:(h + 1) * P],
                                rhs=e_t[:, ci * CHUNK + s * 512:
                                        ci * CHUNK + (s + 1) * 512],
                                start=False,
                                stop=True,
                            )
                        for s in range(2):
                            dcol = h * CHUNK + s * 512
                            if h == 0:
                                nc.vector.tensor_scalar_add(
                                    out=o_sb[:, dcol:dcol + 512],
                                    in0=ps[s][:],
                                    scalar1=b_t[:, 0:1],
                                )
                            else:
                                nc.scalar.add(
                                    out=o_sb[:, dcol:dcol + 512],
                                    in_=ps[s][:],
                                    add=b_t[:, 1:2],
                                )

                    out_view = out_h[:, c0:c0 + CHUNK].rearrange(
                        "(h p) e -> p h e", h=2
                    )
                    nc.sync.dma_start(
                        out=out_view,
                        in_=o_sb[:].rearrange("p (h e) -> p h e", h=2),
                    )

        if reps == 1:
            chunk_body()
        else:
            with tc.For_i(0, reps, 1):
                chunk_body()


def build_nc(n_chunks=N_CHUNKS, low_chunks=LOW_CHUNKS,
             n_nodes_pad=N_NODES_PAD, split=SPLIT, reps=1, n_queues=1):
    per_core = n_chunks * CHUNK

    nc = bacc.Bacc(num_swdge_queues=n_queues)
    handles = (
        nc.declare_dram_parameter("V", [n_nodes_pad, ATOM], BF16, isOutput=False),
        nc.declare_dram_parameter("E", [BOND, per_core], BF16, isOutput=False),
        nc.declare_dram_parameter(
            "idx16", [n_chunks * P, IDX_COLS], I16, isOutput=False
        ),
        nc.declare_dram_parameter("W1", [ATOM, MSG], BF16, isOutput=False),
        nc.declare_dram_parameter("W2", [BOND, MSG], BF16, isOutput=False),
        nc.declare_dram_parameter("b2", [P, 2], F32, isOutput=False),
        nc.declare_dram_parameter("out", [MSG, per_core], BF16, isOutput=True),
    )
    with tile.TileContext(nc) as tc:
        _emit_pipeline(nc, tc, n_chunks, low_chunks, n_nodes_pad, split,
                       handles, reps=reps, n_queues=n_queues)
    return nc


def build_nc_null():
    """Null kernel with identical I/O signature — for RPC/transfer calibration."""
    nc = bacc.Bacc()
    nc.declare_dram_parameter("V", [N_NODES_PAD, ATOM], BF16, isOutput=False)
    nc.declare_dram_parameter("E", [BOND, PER_CORE], BF16, isOutput=False)
    nc.declare_dram_parameter("idx16", [N_CHUNKS * P, IDX_COLS], I16, isOutput=False)
    w1_h = nc.declare_dram_parameter("W1", [ATOM, MSG], BF16, isOutput=False)
    nc.declare_dram_parameter("W2", [BOND, MSG], BF16, isOutput=False)
    nc.declare_dram_parameter("b2", [P, 2], F32, isOutput=False)
    out_h = nc.declare_dram_parameter("out", [MSG, PER_CORE], BF16, isOutput=True)
    with tile.TileContext(nc) as tc:
        with tc.tile_pool(name="p", bufs=1) as pool:
            t = pool.tile([P, MSG], BF16)
            nc.sync.dma_start(out=t[:], in_=w1_h[0:P, :])
            nc.sync.dma_start(out=out_h[0:P, 0:MSG], in_=t[:])
    return nc


_NC_CACHE = {}


def _get_nc(key, **kw):
    if key not in _NC_CACHE:
        nc = (build_nc_null if key == "null" else build_nc)(**kw)
        nc.finalize()  # run Bacc passes (reg alloc, matmul wait legalization)
        _NC_CACHE[key] = nc
    return _NC_CACHE[key]


def _wrap_idx16_all(idx_dev):
    """Vectorized per-chunk gather-index packing for dma_gather.

    Gather position i of a chunk gets the index stored at
    [partition i%16, col i//16], replicated across the 8 16-partition
    bands. Position i lands in output column i (transpose mode).
    """
    a = idx_dev.reshape(N_CHUNKS, CHUNK // 16, 16).transpose(0, 2, 1)  # [nc,16,cols]
    return np.ascontiguousarray(
        np.tile(a, (1, 8, 1)).reshape(N_CHUNKS * P, IDX_COLS).astype(np.int16)
    )


def _make_in_maps(V, E, edge_index, W, b):
    V = np.asarray(V, dtype=np.float32)
    E = np.asarray(E, dtype=np.float32)
    W = np.asarray(W, dtype=np.float32)
    b = np.asarray(b, dtype=np.float32)

    src = np.asarray(edge_index[0]).astype(np.int32)
    v16 = np.zeros((N_NODES_PAD, ATOM), NP_BF16)
    v16[:N_NODES] = V.astype(NP_BF16)
    wt = W.T.astype(NP_BF16)                    # [192, 256]
    w1 = np.ascontiguousarray(wt[0:ATOM])       # [128, 256]
    w2 = np.ascontiguousarray(wt[ATOM:ATOM + BOND])  # [64, 256]
    b2 = np.ascontiguousarray(b.reshape(2, P).T)     # [128, 2]

    in_maps = []
    placements = []
    for i in range(N_CORES):
        lo = i * PER_CORE_RAW
        src_i = src[lo:lo + PER_CORE_RAW]
        e_i = E[lo:lo + PER_CORE_RAW]

        low_pos = np.flatnonzero(src_i < SPLIT)
        high_pos = np.flatnonzero(src_i >= SPLIT)
        n_low, n_high = len(low_pos), len(high_pos)
        assert n_low <= LOW_SLOTS and n_high <= HIGH_SLOTS, (n_low, n_high)

        e_dev = np.zeros((PER_CORE, BOND), np.float32)
        e_dev[:n_low] = e_i[low_pos]
        e_dev[LOW_SLOTS:LOW_SLOTS + n_high] = e_i[high_pos]
        e_t = np.ascontiguousarray(e_dev.T.astype(NP_BF16))  # [64, PER_CORE]

        idx_dev = np.zeros(PER_CORE, np.int32)
        idx_dev[:n_low] = src_i[low_pos]
        idx_dev[LOW_SLOTS:LOW_SLOTS + n_high] = src_i[high_pos] - SPLIT

        in_maps.append(
            {
                "V": v16,
                "E": e_t,
                "idx16": _wrap_idx16_all(idx_dev),
                "W1": w1,
                "W2": w2,
                "b2": b2,
            }
        )
        placements.append((low_pos, high_pos))
    return in_maps, placements


def kernel(V, E, edge_index, W, b):
    in_maps, placements = _make_in_maps(V, E, edge_index, W, b)
    nc = _get_nc("full")
    res = run_bass_kernel_spmd(nc, in_maps, core_ids=list(range(N_CORES)))
    out = np.empty((N_EDGES, MSG), np.float32)
    for i, (low_pos, high_pos) in enumerate(placements):
        dev = res.results[i]["out"]              # [256, PER_CORE] bf16
        dev_f = np.ascontiguousarray(dev.T).astype(np.float32)
        blk = out[i * PER_CORE_RAW:(i + 1) * PER_CORE_RAW]
        blk[low_pos] = dev_f[:len(low_pos)]
        blk[high_pos] = dev_f[LOW_SLOTS:LOW_SLOTS + len(high_pos)]
    return out


def kernel_null(V, E, edge_index, W, b):
    """Calibration: same transfers as kernel(), trivial device work."""
    in_maps, _ = _make_in_maps(V, E, edge_index, W, b)
    nc = _get_nc("null")
    res = run_bass_kernel_spmd(nc, in_maps, core_ids=list(range(N_CORES)))
    return res.results[0]["out"][0, 0]


def kernel_reps(V, E, edge_index, W, b, reps):
    """Timing: identical transfers, device loop repeated `reps` times."""
    in_maps, _ = _make_in_maps(V, E, edge_index, W, b)
    nc = _get_nc(f"reps{reps}", reps=reps)
    res = run_bass_kernel_spmd(nc, in_maps, core_ids=list(range(N_CORES)))
    return res.results[0]["out"][0, 0]


# revision 10
# speedup vs baseline: 103.9500x; 103.9500x over previous
"""Bond-message embedding kernel for TRN2 (8 NeuronCores, SPMD).

Computes out[e, :] = concat(V[src[e]], E[e]) @ W.T + b for 800k edges.

Sharding: edges are globally sorted by source node and split evenly
across the 8 cores, so each core's edges reference a contiguous ~6.3k
node range; W, b replicated; no cross-core comm. All device data is
bf16 (rel-err budget 2e-2; measured ~2.6e-3), which halves HBM traffic
vs f32.

Paired gather: consecutive sorted edges almost always reference the
same or adjacent nodes, so the host ships a per-core paired-V table
  V3[2r]   = [V[r],  V[r]]
  V3[2r+1] = [V[r],  V[r+1]]      (node ids relative to the core range)
and each 512B gather descriptor serves TWO edges. This halves the
descriptor count, keeps descriptors at the 512B line-rate minimum, and
the sorted indices give HBM row locality. ~12.6k table rows per core
means the int16 gather indices need no base-offset splitting.

Compute orientation: out.T = W @ concat(V[src], E).T, so the weights
are the PE's stationary operand (4 LDWEIGHTS per chunk) and the edge
data streams as the moving operand.

Per-core device pipeline (per 1024-edge chunk = 512 pairs):
  1. Load wrapped int16 gather indices [128, 32].
  2. dma_gather(transpose=True, elem_size=256) pulls 512 V3 rows into
     feature-major vt [128, 2*512]: member-0 edges in cols 0:512,
     member-1 edges in cols 512:1024.
  3. E arrives feature-major for free: the host ships E.T [64, per_core]
     bf16 in device-slot order; one [64, 4KB] load covers two chunks.
  4. For msg-half h and edge-slice s (512): PSUM[128,512] = W1h.T-part
     matmul (vt slice) + accumulate W2h.T-part matmul (et slice).
  5. PSUM evac adds the per-partition bias and rounds to bf16 — half 0
     on the vector engine, half 1 on the scalar engine.
  6. One DMA stores both halves [128, 2, 1024] to out.T rows (h*128+p).

The host records slot->edge bookkeeping and reassembles + transposes
the full f32 output.
"""

import math

import ml_dtypes
import numpy as np

import concourse.bacc as bacc
import concourse.mybir as mybir
import concourse.tile as tile
from concourse.bass_utils import run_bass_kernel_spmd

F32 = mybir.dt.float32
BF16 = mybir.dt.bfloat16
I16 = mybir.dt.int16
NP_BF16 = ml_dtypes.bfloat16

N_CORES = 8
N_NODES = 50000
ATOM = 128
BOND = 64
MSG = 256
N_EDGES = 800000

PER_CORE_RAW = N_EDGES // N_CORES   # 100000
CHUNK = 1024                        # edges per chunk
PAIRS = CHUNK // 2                  # gather descriptors per chunk
N_CHUNKS = 98
PER_CORE = N_CHUNKS * CHUNK         # 100352 device slots per core
N_V3 = 13312                        # paired-V table rows (>= 2*(span+1))

P = 128
IDX_COLS = PAIRS // 16              # 32


def _emit_pipeline(nc, tc, n_chunks, handles, reps=1, n_queues=1):
    v_h, e_h, idx_h, w1_h, w2_h, b_h, out_h = handles

    with (
        tc.tile_pool(name="const", bufs=1) as const,
        tc.tile_pool(name="chunkio", bufs=4) as chunkio,
        tc.tile_pool(name="epool", bufs=2) as epool,
        tc.tile_pool(name="work", bufs=2) as work,
        tc.tile_pool(name="psum", bufs=2, space="PSUM") as psum,
    ):
        # --- constants -----------------------------------------------------
        w1t = const.tile([ATOM, MSG], BF16)    # W.T rows 0:128 (atom part)
        nc.sync.dma_start(out=w1t[:], in_=w1_h[:, :])
        w2t = const.tile([BOND, MSG], BF16)    # W.T rows 128:192 (bond part)
        nc.sync.dma_start(out=w2t[:], in_=w2_h[:, :])
        b_t = const.tile([P, 2], F32)          # b_t[p, h] = b[h*128 + p]
        nc.sync.dma_start(out=b_t[:], in_=b_h[:, :])
        # all gather indices stay SBUF-resident (6.3KB/partition)
        idx_all = const.tile([P, n_chunks * IDX_COLS], I16)
        nc.sync.dma_start(out=idx_all[:], in_=idx_h[:, :])

        EG = math.gcd(14, n_chunks)   # chunks per E load
        OG = math.gcd(7, n_chunks)    # chunks per out store

        # --- main loop -----------------------------------------------------
        def chunk_body():
            for c in range(n_chunks):
                if c % EG == 0:
                    e_t = epool.tile([BOND, EG * CHUNK], BF16, tag="et")
                    nc.sync.dma_start(
                        out=e_t[:], in_=e_h[:, c * CHUNK:(c + EG) * CHUNK]
                    )
                if c % OG == 0:
                    o_sb = work.tile([P, 2 * OG * CHUNK], BF16, tag="osb")
                ci, co = c % EG, c % OG

                vt = chunkio.tile([P, CHUNK], BF16, tag="vt")
                nc.gpsimd.dma_gather(
                    out_ap=vt[:].rearrange("p (o n) -> p o n", o=2),
                    in_ap=v_h[:, :],
                    idxs_ap=idx_all[:, c * IDX_COLS:(c + 1) * IDX_COLS],
                    num_idxs=PAIRS,
                    num_idxs_reg=PAIRS,
                    elem_size=2 * ATOM,
                    transpose=True,
                    single_packet=False,
                    queue_num=c % n_queues,
                )

                for h in range(2):
                    ps = [None, None]
                    for s in range(2):
                        ps[s] = psum.tile([P, 512], F32, tag=f"ps{h}{s}",
                                          name=f"ps{h}{s}")
                        nc.tensor.matmul(
                            out=ps[s][:],
                            lhsT=w1t[:, h * P:(h + 1) * P],
                            rhs=vt[:, s * 512:(s + 1) * 512],
                            start=True,
                            stop=False,
                        )
                    for s in range(2):
                        nc.tensor.matmul(
                            out=ps[s][:],
                            lhsT=w2t[:, h * P:(h + 1) * P],
                            rhs=e_t[:, ci * CHUNK + s * 512:
                                    ci * CHUNK + (s + 1) * 512],
                            start=False,
                            stop=True,
                        )
                    for s in range(2):
                        dcol = h * OG * CHUNK + co * CHUNK + s * 512
                        if h == 0:
                            nc.vector.tensor_scalar_add(
                                out=o_sb[:, dcol:dcol + 512],
                                in0=ps[s][:],
                                scalar1=b_t[:, 0:1],
                            )
                        else:
                            nc.scalar.add(
                                out=o_sb[:, dcol:dcol + 512],
                                in_=ps[s][:],
                                add=b_t[:, 1:2],
                            )

                if co == OG - 1:
                    c0 = (c - co) * CHUNK
                    out_view = out_h[:, c0:c0 + OG * CHUNK].rearrange(
                        "(h p) e -> p h e", h=2
                    )
                    nc.sync.dma_start(
                        out=out_view,
                        in_=o_sb[:].rearrange("p (h e) -> p h e", h=2),
                    )

        if reps == 1:
            chunk_body()
        else:
            with tc.For_i(0, reps, 1):
                chunk_body()


def build_nc(n_chunks=N_CHUNKS, n_v3=N_V3, reps=1, n_queues=1):
    per_core = n_chunks * CHUNK

    nc = bacc.Bacc(num_swdge_queues=n_queues)
    handles = (
        nc.declare_dram_parameter("V3", [n_v3, 2 * ATOM], BF16, isOutput=False),
        nc.declare_dram_parameter("E", [BOND, per_core], BF16, isOutput=False),
        nc.declare_dram_parameter(
            "idx16", [P, n_chunks * IDX_COLS], I16, isOutput=False
        ),
        nc.declare_dram_parameter("W1", [ATOM, MSG], BF16, isOutput=False),
        nc.declare_dram_parameter("W2", [BOND, MSG], BF16, isOutput=False),
        nc.declare_dram_parameter("b2", [P, 2], F32, isOutput=False),
        nc.declare_dram_parameter("out", [MSG, per_core], BF16, isOutput=True),
    )
    with tile.TileContext(nc) as tc:
        _emit_pipeline(nc, tc, n_chunks, handles, reps=reps, n_queues=n_queues)
    return nc


def build_nc_null():
    """Null kernel with identical I/O signature — for RPC/transfer calibration."""
    nc = bacc.Bacc()
    nc.declare_dram_parameter("V3", [N_V3, 2 * ATOM], BF16, isOutput=False)
    nc.declare_dram_parameter("E", [BOND, PER_CORE], BF16, isOutput=False)
    nc.declare_dram_parameter("idx16", [P, N_CHUNKS * IDX_COLS], I16, isOutput=False)
    w1_h = nc.declare_dram_parameter("W1", [ATOM, MSG], BF16, isOutput=False)
    nc.declare_dram_parameter("W2", [BOND, MSG], BF16, isOutput=False)
    nc.declare_dram_parameter("b2", [P, 2], F32, isOutput=False)
    out_h = nc.declare_dram_parameter("out", [MSG, PER_CORE], BF16, isOutput=True)
    with tile.TileContext(nc) as tc:
        with tc.tile_pool(name="p", bufs=1) as pool:
            t = pool.tile([P, MSG], BF16)
            nc.sync.dma_start(out=t[:], in_=w1_h[0:P, :])
            nc.sync.dma_start(out=out_h[0:P, 0:MSG], in_=t[:])
    return nc


_NC_CACHE = {}


def _get_nc(key, **kw):
    if key not in _NC_CACHE:
        nc = (build_nc_null if key == "null" else build_nc)(**kw)
        nc.finalize()  # run Bacc passes (reg alloc, matmul wait legalization)
        _NC_CACHE[key] = nc
    return _NC_CACHE[key]


def _wrap_idx16_all(rows, n_chunks):
    """Per-chunk gather-index packing for dma_gather.

    Gather position i of a chunk reads the index at [partition i%16,
    col i//16], replicated across the 8 16-partition bands. Position i
    lands in output columns i (member 0) and 512+i (member 1).
    """
    a = rows.reshape(n_chunks, IDX_COLS, 16).transpose(0, 2, 1)  # [nc,16,cols]
    a = np.tile(a, (1, 8, 1))                # [nc, 128, cols]
    return np.ascontiguousarray(
        a.transpose(1, 0, 2).reshape(P, n_chunks * IDX_COLS).astype(np.int16)
    )


def _pack_core(sl, s):
    """Greedy pairing of the sorted edge list of one core.

    Returns (pair_rows [n_pairs], edge_ids [n_pairs, 2] with -1 dummies,
    node_lo, span).
    """
    node_lo = int(s[0])
    rel = s - node_lo
    n = len(s)
    pair_rows = np.empty(n, np.int32)
    edge_ids = np.full((n, 2), -1, np.int64)
    k = 0
    np_pairs = 0
    while k < n:
        r = int(rel[k])
        if k + 1 < n and rel[k + 1] - r <= 1:
            pair_rows[np_pairs] = 2 * r + int(rel[k + 1] - r)
            edge_ids[np_pairs, 0] = sl[k]
            edge_ids[np_pairs, 1] = sl[k + 1]
            k += 2
        else:
            pair_rows[np_pairs] = 2 * r
            edge_ids[np_pairs, 0] = sl[k]
            k += 1
        np_pairs += 1
    span = int(rel[-1]) + 1
    return pair_rows[:np_pairs], edge_ids[:np_pairs], node_lo, span


def _make_in_maps(V, E, edge_index, W, b):
    V = np.asarray(V, dtype=np.float32)
    E = np.asarray(E, dtype=np.float32)
    W = np.asarray(W, dtype=np.float32)
    b = np.asarray(b, dtype=np.float32)
    src = np.asarray(edge_index[0]).astype(np.int64)

    order = np.argsort(src, kind="stable")
    wt = W.T.astype(NP_BF16)                         # [192, 256]
    w1 = np.ascontiguousarray(wt[0:ATOM])            # [128, 256]
    w2 = np.ascontiguousarray(wt[ATOM:ATOM + BOND])  # [64, 256]
    b2 = np.ascontiguousarray(b.reshape(2, P).T)     # [128, 2]
    v16 = V.astype(NP_BF16)

    in_maps = []
    placements = []
    for i in range(N_CORES):
        sl = order[i * PER_CORE_RAW:(i + 1) * PER_CORE_RAW]
        s = src[sl]
        pair_rows, edge_ids, node_lo, span = _pack_core(sl, s)
        n_pairs = len(pair_rows)
        assert n_pairs <= N_CHUNKS * PAIRS, n_pairs
        assert 2 * (span + 1) <= N_V3, span

        v3 = np.zeros((N_V3, 2 * ATOM), NP_BF16)
        vr = v16[node_lo:min(node_lo + span + 1, N_NODES)]
        nr = vr.shape[0]
        v3[0:2 * span:2, 0:ATOM] = vr[:span]
        v3[0:2 * span:2, ATOM:] = vr[:span]
        v3[1:2 * span:2, 0:ATOM] = vr[:span]
        v3[1:2 * (nr - 1):2, ATOM:] = vr[1:nr]

        rows = np.zeros(N_CHUNKS * PAIRS, np.int32)
        rows[:n_pairs] = pair_rows
        ee = np.full((N_CHUNKS * PAIRS, 2), -1, np.int64)
        ee[:n_pairs] = edge_ids

        # device slot of pair i member j in chunk c: c*1024 + j*512 + i%512
        pi = np.arange(N_CHUNKS * PAIRS)
        slot = (pi // PAIRS) * CHUNK + (pi % PAIRS)  # member-0 slot
        slots2 = np.stack([slot, slot + PAIRS], axis=1)  # [pairs, 2]

        e_dev = np.zeros((PER_CORE, BOND), np.float32)
        mask = ee >= 0
        e_dev[slots2[mask]] = E[ee[mask]]
        e_t = np.ascontiguousarray(e_dev.T.astype(NP_BF16))  # [64, PER_CORE]

        in_maps.append(
            {
                "V3": v3,
                "E": e_t,
                "idx16": _wrap_idx16_all(rows, N_CHUNKS),
                "W1": w1,
                "W2": w2,
                "b2": b2,
            }
        )
        placements.append((ee[mask], slots2[mask]))
    return in_maps, placements


def kernel(V, E, edge_index, W, b):
    in_maps, placements = _make_in_maps(V, E, edge_index, W, b)
    nc = _get_nc("full")
    res = run_bass_kernel_spmd(nc, in_maps, core_ids=list(range(N_CORES)))
    out = np.empty((N_EDGES, MSG), np.float32)
    for i, (eids, slots) in enumerate(placements):
        dev = res.results[i]["out"]              # [256, PER_CORE] bf16
        dev_f = np.ascontiguousarray(dev.T).astype(np.float32)
        out[eids] = dev_f[slots]
    return out


def kernel_null(V, E, edge_index, W, b):
    """Calibration: same transfers as kernel(), trivial device work."""
    in_maps, _ = _make_in_maps(V, E, edge_index, W, b)
    nc = _get_nc("null")
    res = run_bass_kernel_spmd(nc, in_maps, core_ids=list(range(N_CORES)))
    return res.results[0]["out"][0, 0]


def kernel_reps(V, E, edge_index, W, b, reps):
    """Timing: identical transfers, device loop repeated `reps` times."""
    in_maps, _ = _make_in_maps(V, E, edge_index, W, b)
    nc = _get_nc(f"reps{reps}", reps=reps)
    res = run_bass_kernel_spmd(nc, in_maps, core_ids=list(range(N_CORES)))
    return res.results[0]["out"][0, 0]
